# revision 1
# baseline (speedup 1.0000x reference)
"""Trainium2 Bass kernel for GQA attention (B=2, S=1024, HID=4096, H=32,
HKV=8, HD=128) with NeoX rotary + additive mask, sharded over 8 NeuronCores.

Sharding: 2 data-parallel groups (one per batch sequence) x 4-way tensor
parallel (8 q-heads / 2 kv-heads per core). wq/wk/wv column-sharded,
wo row-sharded; ReduceScatter(add) within each 4-core group after wo, and the
host concatenates the 8 disjoint row shards.

Everything on device runs in a transposed layout ([feature, token]) so every
matmul streams with free-dim 512 at full PE rate (fp32r for attention/wo,
bf16 inputs for the QKV projections with fp32 PSUM accumulation).
"""

import math

import ml_dtypes
import numpy as np

B, S, HID, H, HKV, HD = 2, 1024, 4096, 32, 8, 128
NCORES = 8
TPG = 4                      # tensor-parallel group size
NGROUPS = NCORES // TPG      # data-parallel groups (= B)
HL = H // TPG                # q heads per core (8)
KVL = HKV // TPG             # kv heads per core (2)
GQ = H // HKV                # q heads per kv head (4)
SCALE = 1.0 / math.sqrt(HD)
QB = 512                     # q block (free dim of attention matmuls)
NEG_THRESH = -1.0e8          # mask values <= this count as fully masked
# ReduceScatter chunk boundaries in 128-token tiles (device + host unshard)
RS_CHUNKS = [(0, 2), (2, 4), (4, 8)]

_STATE: dict = {}


# ----------------------------------------------------------------------------
# walrus compat: this toolchain supports at most ONE semaphore wait per
# instruction; Tile's scheduler can attach several. Hoist extras onto
# same-engine nops placed immediately before the instruction.
# ----------------------------------------------------------------------------
def _split_multi_waits(nc):
    import concourse.mybir as mybir

    def detached_nop(engine_type):
        bi = nc.engines[engine_type].nop()
        inst = bi.ins
        for fn in nc.m.functions:
            for b in fn.blocks:
                il = b.instructions
                if il and il[-1].name == inst.name:
                    il.pop()
                    return inst
        raise AssertionError("could not detach nop")

    for fn in nc.m.functions:
        for b in fn.blocks:
            il = b.instructions
            out = []
            changed = False
            for inst in il:
                si = inst.sync_info
                waits = list(si.on_wait) if (si is not None and si.on_wait) else []
                if len(waits) > 1:
                    for w in waits[:-1]:
                        nop = detached_nop(inst.engine)
                        nop.sync_info = mybir.SyncInfo(on_wait=[w], on_update=[])
                        out.append(nop)
                    si.on_wait = waits[-1:]
                    changed = True
                out.append(inst)
            if changed:
                b.instructions = out


# ----------------------------------------------------------------------------
# Device program
# ----------------------------------------------------------------------------
def _build_module(mask_desc):
    """mask_desc: per (qb, kb) block descriptor list computed on the host from
    the actual attn_mask:
      ("skip",)                 block fully masked
      ("full", need_mask:bool)  full 512-wide block, optionally + mask data
      ("causal", off:int)       causal window: cols [off,512) active, mask
                                add on the 128-wide diagonal window at `off`
    """
    import concourse.bass as bass
    import concourse.mybir as mybir
    import concourse.tile as tile
    from concourse.masks import make_identity

    dt = mybir.dt
    f32, f32r, bf16 = dt.float32, dt.float32r, dt.bfloat16
    KT = HID // 128  # 32 contraction tiles

    nc = bass.Bass()

    # --- DRAM parameters (per-core shards, host-prepared) ---
    xt_in = nc.declare_dram_parameter("xt", [S // QB, KT, 128, QB], bf16,
                                      isOutput=False)
    wq_in = nc.declare_dram_parameter("wq", [HL, 128, KT, 128], bf16, isOutput=False)
    wk_in = nc.declare_dram_parameter("wk", [KVL, 128, KT, 128], bf16, isOutput=False)
    wv_in = nc.declare_dram_parameter("wv", [KVL, 128, KT, 128], bf16, isOutput=False)
    wo_in = nc.declare_dram_parameter("wo", [HL, 128, HID], bf16, isOutput=False)
    cos_in = nc.declare_dram_parameter("cos_t", [128, S], f32, isOutput=False)
    sin_in = nc.declare_dram_parameter("sin_t", [128, S], f32, isOutput=False)
    # mask blocks actually referenced by the program, in transposed [kv, q]
    # layout; index map built below.
    mask_tiles = []
    for qb in range(S // QB):
        for kb in range(S // 128):
            d = mask_desc[qb][kb]
            if d[0] == "full" and d[1]:
                mask_tiles.append((qb, kb, QB))
    nmask = max(1, len(mask_tiles))
    mw = max([t[2] for t in mask_tiles], default=128)
    mask_in = nc.declare_dram_parameter("maskt", [nmask, 128, mw], f32, isOutput=False)
    tri_in = nc.declare_dram_parameter("tri01", [128, 128], bf16, isOutput=False)
    out_ext = nc.declare_dram_parameter("outp", [S // TPG, HID], bf16, isOutput=True)

    from contextlib import ExitStack
    ctx = ExitStack()
    with tile.TileContext(nc) as tc:
        const = ctx.enter_context(tc.tile_pool(name="const", bufs=1))
        persist = ctx.enter_context(tc.tile_pool(name="persist", bufs=1))
        dram = ctx.enter_context(tc.tile_pool(name="dram", bufs=1, space="DRAM"))
        qkvpool = ctx.enter_context(tc.tile_pool(name="qkv", bufs=1))

        # qb1 is processed FIRST so its (big) RS chunk overlaps qb0's
        # compute; qb0's two smaller chunks fire as its wo tiles finish,
        # shortening the end-of-kernel collective tail.
        rs_in = dram.tile([S, HID], bf16)
        rs_out = [dram.tile([(t1 - t0) * 128 // TPG, HID], bf16,
                            name=f"rs_out{ci}")
                  for ci, (t0, t1) in enumerate(RS_CHUNKS)]

        cos_t = const.tile([128, S], f32, tag="cos")
        sin_t = const.tile([128, S], f32, tag="sin")
        nc.sync.dma_start(out=cos_t[:], in_=cos_in[:])
        nc.sync.dma_start(out=sin_t[:], in_=sin_in[:])
        ones32 = const.tile([128, 128], f32, tag="ones32")
        nc.gpsimd.memset(ones32[:], 1.0)
        ones_t = const.tile([128, 128], bf16, tag="ones")
        nc.vector.tensor_copy(ones_t[:], ones32[:])
        ident = const.tile([128, 128], f32, tag="ident")
        make_identity(nc, ident[:])
        mask_sb = const.tile([128, nmask, mw], f32, tag="mask")
        nc.sync.dma_start(out=mask_sb[:], in_=mask_in[:].rearrange("b p c -> p b c"))
        mask_idx = {(qb, kb): i for i, (qb, kb, _) in enumerate(mask_tiles)}
        tri01 = const.tile([128, 128], bf16, tag="tri01")
        nc.sync.dma_start(out=tri01[:], in_=tri_in[:])

        # activations that live through phase 2 (freed before phase 3)
        q_rot = [qkvpool.tile([128, S], f32r, tag=f"q{h}", name=f"q_rot{h}")
                 for h in range(HL)]
        k_rot = [qkvpool.tile([128, S], f32r, tag=f"k{j}", name=f"k_rot{j}")
                 for j in range(KVL)]
        v_nat = [qkvpool.tile([128, S // 128, 128], bf16, tag=f"v{j}", name=f"v_nat{j}")
                 for j in range(KVL)]

        # ------- attention machinery (used both fused into phase 1 for qb1
        # and standalone for qb0) -------
        attn = persist.tile([128, HL, S], bf16, tag="attn")
        NHB = HID // QB  # 8 hid blocks
        ppool = ctx.enter_context(tc.tile_pool(name="p2p", bufs=3))
        rpool = ctx.enter_context(tc.tile_pool(name="p2r", bufs=2))

        def head_blocks(qb):
            blocks = []
            for kb in range(S // 128):
                d = mask_desc[qb][kb]
                if d[0] == "skip":
                    continue
                if d[0] == "causal":
                    blocks.append((kb, d[1], ("diag", d[1])))
                else:
                    blocks.append((kb, 0, ("full",) if d[1] else None))
            return blocks

        def make_attention(qb, scpool, pvpool, dnpool):
            blocks = head_blocks(qb)
            nblk = len(blocks)
            state = {}  # h -> (ps_pv, ps_dn, p_ts)

            def issue_score(h, bi):
                kvh = h // GQ
                kb, off, mk = blocks[bi]
                qsl = slice(qb * QB + off, (qb + 1) * QB)
                ps_sc = scpool.tile([128, QB], f32, tag="ps_sc")
                nc.tensor.matmul(
                    ps_sc[:, off:QB],
                    k_rot[kvh][:, kb * 128:(kb + 1) * 128],
                    q_rot[h][:, qsl],
                    start=True, stop=True,
                )
                if mk is not None and mk[0] != "diag":
                    # rare general path: additive mask on PSUM via DVE
                    mi = mask_idx[(qb, kb)]
                    nc.vector.tensor_tensor(
                        ps_sc[:, 0:QB], ps_sc[:, 0:QB],
                        mask_sb[:, mi, 0:QB], mybir.AluOpType.add)
                p_t = ppool.tile([128, QB], bf16, tag="p")
                nc.scalar.activation(
                    p_t[:, off:QB], ps_sc[:, off:QB],
                    mybir.ActivationFunctionType.Exp)
                if mk is not None and mk[0] == "diag":
                    # causal diagonal: zero the kv>q triangle of exp(s) in
                    # SBUF on the (otherwise idle) Pool engine
                    nc.gpsimd.tensor_tensor(
                        p_t[:, off:off + 128], p_t[:, off:off + 128],
                        tri01[:], mybir.AluOpType.mult)
                if h not in state:
                    ps_pv = pvpool.tile([128, QB], f32, tag="ps_pv",
                                        name=f"ps_pv{qb}_{h}")
                    ps_dn = dnpool.tile([128, QB], f32, tag="ps_dn",
                                        name=f"ps_dn{qb}_{h}")
                    state[h] = (ps_pv, ps_dn, {})
                state[h][2][bi] = p_t

            def issue_pv_dn(h, bi):
                kvh = h // GQ
                kb, off, mk = blocks[bi]
                ps_pv, ps_dn, p_ts = state[h]
                p_t = p_ts.pop(bi)
                nc.tensor.matmul(
                    ps_pv[:, off:QB],
                    v_nat[kvh][:, kb, :],
                    p_t[:, off:QB],
                    start=(bi == 0), stop=(bi == nblk - 1),
                )
                nc.tensor.matmul(
                    ps_dn[:, off:QB],
                    ones_t[:],
                    p_t[:, off:QB],
                    start=(bi == 0), stop=(bi == nblk - 1),
                )
                if bi == nblk - 1:
                    recip = rpool.tile([128, QB], f32, tag="recip")
                    nc.vector.reciprocal(recip[:], ps_dn[:])
                    nc.vector.tensor_tensor(
                        attn[:, h, qb * QB:(qb + 1) * QB], ps_pv[:],
                        recip[:], mybir.AluOpType.mult)

            return nblk, issue_score, issue_pv_dn

        # ------ phase 1 (k/v/q projections), with qb1's attention fused
        # between the q chains: each head's cross-engine latencies (exp,
        # tri-zero, normalize) hide behind the following 6.8us q chain.
        with tc.tile_pool(name="p1x", bufs=1) as xpool, \
             tc.tile_pool(name="p1w", bufs=3) as wpool, \
             tc.tile_pool(name="p1t", bufs=2) as tpool, \
             tc.tile_pool(name="p1ps", bufs=2, space="PSUM") as pspool:

            # first weight tile DMA'd before x so the first chain's lhsT is
            # never the long pole
            w0_sb = wpool.tile([128, KT, 128], bf16, tag="w")
            nc.sync.dma_start(out=w0_sb[:], in_=wk_in[0])

            # one tile per (tb, kt-chunk) so the first projection chain only
            # waits on the tb=0 quarter-chunks, in DMA issue order.
            xt = [[xpool.tile([128, KT // 4, QB], bf16, tag=f"xt{tb}_{i}",
                              name=f"xt{tb}_{i}") for i in range(4)]
                  for tb in range(S // QB)]
            for tb in range(S // QB):
                for i in range(4):
                    nc.sync.dma_start(
                        out=xt[tb][i][:],
                        in_=xt_in[tb, i * (KT // 4):(i + 1) * (KT // 4), :, :]
                            .rearrange("k p t -> p k t"),
                    )

            def xt_sl(kt, tb):
                return xt[tb][kt // (KT // 4)][:, kt % (KT // 4), :]

            w_drams = {"k": wk_in, "v": wv_in, "q": wq_in}
            w_tiles = {}

            def chain(kind, ct, tb, pstr=None):
                if tb == 0:
                    if kind == "k" and ct == 0:
                        w_tiles["cur"] = w0_sb
                    else:
                        t = wpool.tile([128, KT, 128], bf16, tag="w",
                                       name=f"w_{kind}{ct}")
                        nc.sync.dma_start(out=t[:], in_=w_drams[kind][ct])
                        w_tiles["cur"] = t
                w_sb = w_tiles["cur"]
                ps = pspool.tile([128, QB], f32, tag="ps_qkv")
                for kt in range(KT):
                    nc.tensor.matmul(
                        ps[:],
                        w_sb[:, kt, :],
                        xt_sl(kt, tb),
                        start=(kt == 0),
                        stop=(kt == KT - 1),
                    )
                tsl = slice(tb * QB, (tb + 1) * QB)
                if kind in ("q", "k"):
                    dest = q_rot[ct] if kind == "q" else k_rot[ct]
                    swap = tpool.tile([128, QB], f32, tag="swap")
                    nc.scalar.activation(
                        swap[0:64, :], ps[64:128, :],
                        mybir.ActivationFunctionType.Copy, scale=-1.0)
                    nc.scalar.activation(
                        swap[64:128, :], ps[0:64, :],
                        mybir.ActivationFunctionType.Copy)
                    t2 = tpool.tile([128, QB], f32, tag="t2")
                    nc.vector.tensor_tensor(
                        t2[:], ps[:], cos_t[:, tsl], mybir.AluOpType.mult)
                    t3 = tpool.tile([128, QB], f32, tag="t3")
                    nc.vector.tensor_tensor(
                        t3[:], swap[:], sin_t[:, tsl], mybir.AluOpType.mult)
                    nc.vector.tensor_tensor(
                        dest[:, tsl], t2[:], t3[:], mybir.AluOpType.add)
                else:  # v: transpose to natural [t, d] layout
                    vt = tpool.tile([128, QB], f32, tag="vt")
                    nc.scalar.activation(
                        vt[:], ps[:], mybir.ActivationFunctionType.Copy)
                    for j in range(QB // 128):
                        ps_t = pstr.tile([128, 128], f32, tag="ps_tr")
                        nc.tensor.transpose(
                            ps_t[:], vt[:, j * 128:(j + 1) * 128], ident[:])
                        nc.vector.tensor_copy(
                            v_nat[ct][:, tb * (QB // 128) + j, :], ps_t[:])

            # k/v chains first (tb-interleaved so x-feed stalls stay short)
            with tc.tile_pool(name="p1pst", bufs=2, space="PSUM") as pstr:
                for kind, n in (("k", KVL), ("v", KVL)):
                    for ct in range(n):
                        for tb in range(S // QB):
                            chain(kind, ct, tb, pstr)

            # q chains (plain)
            for ct in range(HL):
                for tb in range(S // QB):
                    chain("q", ct, tb)

        # ------- attention(qb1), wo(qb1), attention(qb0), wo(qb0); wo_sb
        # prefetched under qb1's attention -------
        wopool = ctx.enter_context(tc.tile_pool(name="p23w", bufs=1))
        opool = ctx.enter_context(tc.tile_pool(name="p3o", bufs=4))
        wo_sb = wopool.tile([128, HL, HID], bf16, tag="wo")
        nc.sync.dma_start(
            out=wo_sb[:], in_=wo_in[:, :, :].rearrange("c p n -> p c n"))

        def attention_phase(qb, scp, pvp, dnp):
            nblk, score, pvdn = make_attention(qb, scp, pvp, dnp)
            events = [(h, bi) for h in range(HL) for bi in range(nblk)]
            LOOKAHEAD = 1
            n = len(events)
            for j in range(min(LOOKAHEAD, n)):
                score(*events[j])
            for j in range(LOOKAHEAD, n):
                score(*events[j])
                pvdn(*events[j - LOOKAHEAD])
            for j in range(max(0, n - LOOKAHEAD), n):
                pvdn(*events[j])

        def wo_phase(qb, pso):
            for tt in range(qb * 4, qb * 4 + 4):
                for hb in range(NHB):
                    ps_o = pso.tile([128, QB], f32, tag="ps_o")
                    for ct in range(HL):
                        nc.tensor.matmul(
                            ps_o[:],
                            attn[:, ct, tt * 128:(tt + 1) * 128],
                            wo_sb[:, ct, hb * QB:(hb + 1) * QB],
                            start=(ct == 0), stop=(ct == HL - 1),
                        )
                    o_sb = opool.tile([128, QB], bf16, tag="o")
                    if (hb + tt) % 2 == 0:
                        nc.vector.tensor_copy(o_sb[:], ps_o[:])
                    else:
                        nc.scalar.activation(
                            o_sb[:], ps_o[:], mybir.ActivationFunctionType.Copy)
                    nc.sync.dma_start(
                        out=rs_in[tt * 128:(tt + 1) * 128,
                                  hb * QB:(hb + 1) * QB],
                        in_=o_sb[:])

                # reduce-scatter each finished chunk of partial outputs
                # across the TP group while later compute proceeds; core
                # r gets the r-th quarter of each chunk's token rows.
                # The bounce copy to out_ext is issued from the gpsimd
                # queue (collectives only): on the SP queue its wait on
                # the collective would block all later wo DMAs.
                for ci, (t0, t1) in enumerate(RS_CHUNKS):
                    if tt + 1 != t1:
                        continue
                    rows = (t1 - t0) * 128 // TPG
                    r0 = t0 * 128 // TPG
                    nc.gpsimd.collective_compute(
                        "ReduceScatter", mybir.AluOpType.add,
                        replica_groups=[list(range(g * TPG, (g + 1) * TPG))
                                        for g in range(NGROUPS)],
                        ins=[rs_in[t0 * 128:t1 * 128, :].opt()],
                        outs=[rs_out[ci].opt()],
                    )
                    nc.sync.dma_start(
                        out=out_ext[r0:r0 + rows, :],
                        in_=rs_out[ci][:])

        for qb in (1, 0):
            with tc.tile_pool(name=f"p2sc{qb}", bufs=3, space="PSUM") as scp, \
                 tc.tile_pool(name=f"p2pv{qb}", bufs=2, space="PSUM") as pvp, \
                 tc.tile_pool(name=f"p2dn{qb}", bufs=2, space="PSUM") as dnp:
                attention_phase(qb, scp, pvp, dnp)
            with tc.tile_pool(name=f"p3ps{qb}", bufs=3, space="PSUM") as psp:
                wo_phase(qb, psp)



        ctx.close()

    _split_multi_waits(nc)
    return nc, [t[:2] for t in mask_tiles], mw


# ----------------------------------------------------------------------------
# Host-side input prep
# ----------------------------------------------------------------------------
def _classify_mask(attn_mask):
    """Per (qb, kb) descriptor from the actual mask contents (transposed
    [kv, q] view). Causal masks produce the efficient windowed structure."""
    mt = attn_mask.T  # [kv, q]
    desc = []
    for qb in range(S // QB):
        row = []
        q0 = qb * QB
        for kb in range(S // 128):
            blk = mt[kb * 128:(kb + 1) * 128, q0:q0 + QB]
            if np.all(blk <= NEG_THRESH):
                row.append(("skip",))
                continue
            if np.all(np.abs(blk) < 1e-6):
                row.append(("full", False))
                continue
            # causal window? cols [0, off) fully masked, diag at [off, off+128),
            # cols beyond fully visible
            off = kb * 128 - q0
            causal = False
            if 0 <= off <= QB - 128:
                left_ok = np.all(blk[:, :off] <= NEG_THRESH) if off else True
                right_ok = (np.all(np.abs(blk[:, off + 128:]) < 1e-6)
                            if off + 128 < QB else True)
                causal = bool(left_ok and right_ok)
            if causal:
                row.append(("causal", off))
            else:
                row.append(("full", True))
        desc.append(row)
    # every q column must keep at least one contributing block
    for qb in range(S // QB):
        assert any(d[0] != "skip" for d in desc[qb]), "fully-masked q rows unsupported"
    return desc


def _prep_core_inputs(inputs, mask_desc, mask_list, mw):
    x = np.asarray(inputs["x"], np.float32)
    wq = np.asarray(inputs["wq"], np.float32)
    wk = np.asarray(inputs["wk"], np.float32)
    wv = np.asarray(inputs["wv"], np.float32)
    wo = np.asarray(inputs["wo"], np.float32)
    attn_mask = np.asarray(inputs["attn_mask"], np.float32)
    start_pos = np.asarray(inputs["start_pos"], np.int32)

    bf = ml_dtypes.bfloat16
    KT = HID // 128

    inv_freq = 1.0 / (10000.0 ** (np.arange(0, HD, 2, dtype=np.float32) / HD))
    mt = attn_mask.T
    if mask_list:
        mask_arr = np.zeros((len(mask_list), 128, mw), np.float32)
        for i, (qb, kb) in enumerate(mask_list):
            mask_arr[i, :, 0:QB] = mt[kb * 128:(kb + 1) * 128,
                                      qb * QB:(qb + 1) * QB]
    else:
        mask_arr = np.zeros((1, 128, mw), np.float32)
    # 0/1 lower-triangle (kv <= q) pattern shared by every causal diag block
    tri01 = (np.arange(128)[:, None] <= np.arange(128)[None, :]).astype(bf)

    # lhsT tile layout: [ct, p=hid_within_kt, kt, col_within_ct]
    def wtile2(w):
        c = w.shape[1]
        return np.ascontiguousarray(
            w.reshape(KT, 128, c // 128, 128).transpose(2, 1, 0, 3))

    in_maps = []
    for core in range(NCORES):
        g, r = divmod(core, TPG)
        xb = x[g * S:(g + 1) * S]                       # [S, HID]
        xt = np.ascontiguousarray(
            xb.T.reshape(KT, 128, S // QB, QB).transpose(2, 0, 1, 3)).astype(bf)
        wq_c = (wq[:, r * HL * HD:(r + 1) * HL * HD] * SCALE)
        wk_c = wk[:, r * KVL * HD:(r + 1) * KVL * HD]
        wv_c = wv[:, r * KVL * HD:(r + 1) * KVL * HD]
        wo_c = wo[r * HL * HD:(r + 1) * HL * HD, :]     # [1024, HID]

        pos = start_pos[g] + np.arange(S, dtype=np.float32)
        ang = pos[:, None] * inv_freq[None, :]          # [S, HD/2]
        cos = np.concatenate([np.cos(ang), np.cos(ang)], -1).T  # [HD, S]
        sin = np.concatenate([np.sin(ang), np.sin(ang)], -1).T

        in_maps.append({
            "xt": xt,
            "wq": wtile2(wq_c).astype(bf),
            "wk": wtile2(wk_c).astype(bf),
            "wv": wtile2(wv_c).astype(bf),
            "wo": np.ascontiguousarray(wo_c.reshape(HL, 128, HID)).astype(bf),
            "cos_t": np.ascontiguousarray(cos.astype(np.float32)),
            "sin_t": np.ascontiguousarray(sin.astype(np.float32)),
            "maskt": mask_arr,
            "tri01": tri01,
        })
    return in_maps


def _make_runner(nc):
    """Cached jit over the bass module (adapted from
    concourse.bass2jax.run_bass_via_pjrt so repeat calls reuse one NEFF)."""
    import jax
    import jax.numpy as jnp
    from jax.sharding import Mesh, NamedSharding, PartitionSpec
    from jax.experimental.shard_map import shard_map

    import concourse.mybir as mybir
    from concourse import bass2jax

    bass2jax.install_neuronx_cc_hook()
    assert nc.dbg_addr is None
    partition_name = (nc.partition_id_tensor.name
                      if nc.partition_id_tensor else None)

    in_names, out_names, out_avals, out_shapes = [], [], [], []
    for alloc in nc.m.functions[0].allocations:
        if not isinstance(alloc, mybir.MemoryLocationSet):
            continue
        name = alloc.memorylocations[0].name
        if alloc.kind == "ExternalInput":
            if name != partition_name:
                in_names.append(name)
        elif alloc.kind == "ExternalOutput":
            assert alloc.tensor_shape is not None and alloc.dtype is not None
            shape = tuple(alloc.tensor_shape)
            npdt = mybir.dt.np(alloc.dtype)
            out_names.append(name)
            out_shapes.append((shape, npdt))
            out_avals.append(jax.core.ShapedArray(shape, npdt))

    n_params = len(in_names)
    n_outs = len(out_names)
    all_in_names = in_names + out_names
    if partition_name is not None:
        all_in_names = all_in_names + [partition_name]
    donate = tuple(range(n_params, n_params + n_outs))

    def _body(*args):
        operands = list(args)
        if partition_name is not None:
            operands.append(bass2jax.partition_id_tensor())
        outs = bass2jax._bass_exec_p.bind(
            *operands,
            out_avals=tuple(out_avals),
            in_names=tuple(all_in_names),
            out_names=tuple(out_names),
            lowering_input_output_aliases=(),
            sim_require_finite=True,
            sim_require_nnan=True,
            nc=nc,
        )
        return tuple(outs)

    devices = jax.devices()[:NCORES]
    mesh = Mesh(np.asarray(devices), ("core",))
    pc = PartitionSpec("core")
    sharded = jax.jit(
        shard_map(_body, mesh=mesh, in_specs=(pc,) * (n_params + n_outs),
                  out_specs=(pc,) * n_outs, check_rep=False),
        donate_argnums=donate, keep_unused=True)

    shard_dev = NamedSharding(mesh, pc)

    def make_zeros():
        return tuple(
            jax.device_put(np.zeros((NCORES * s[0], *s[1:]), d), shard_dev)
            for s, d in out_shapes)

    def put_inputs(in_maps):
        return [
            jax.device_put(
                np.concatenate([np.asarray(m[nm]) for m in in_maps], axis=0),
                shard_dev)
            for nm in in_names]

    def run_from_dev(in_dev, zeros):
        out_arrs = sharded(*in_dev, *zeros)
        jax.block_until_ready(out_arrs)
        return out_arrs

    def run(in_maps):
        out_arrs = run_from_dev(put_inputs(in_maps), make_zeros())
        return [
            {nm: np.asarray(out_arrs[i]).reshape(NCORES, *out_shapes[i][0])[c]
             for i, nm in enumerate(out_names)}
            for c in range(NCORES)]

    return {"run": run, "put_inputs": put_inputs, "make_zeros": make_zeros,
            "run_from_dev": run_from_dev, "sharded": sharded}


def _get_runner(mask_desc):
    key = repr(mask_desc)
    if _STATE.get("key") == key:
        return _STATE["run"], _STATE["mask_list"], _STATE["mw"]

    nc, mask_list, mw = _build_module(mask_desc)
    runner = _make_runner(nc)

    _STATE.update({"key": key, "run": runner["run"], "mask_list": mask_list,
                   "mw": mw, "nc": nc, "runner": runner})
    return runner["run"], mask_list, mw


def kernel(**inputs) -> np.ndarray:
    attn_mask = np.asarray(inputs["attn_mask"], np.float32)
    mask_desc = _classify_mask(attn_mask)
    run, mask_list, mw = _get_runner(mask_desc)
    in_maps = _prep_core_inputs(inputs, mask_desc, mask_list, mw)
    results = run(in_maps)
    out = np.empty((B * S, HID), np.float32)
    for core in range(NCORES):
        g, r = divmod(core, TPG)
        # per RS chunk, core r holds the r-th quarter of the chunk's tokens
        res = np.asarray(results[core]["outp"], np.float32)
        for t0, t1 in RS_CHUNKS:
            rows = (t1 - t0) * 128 // TPG
            src0 = t0 * 128 // TPG
            dst0 = g * S + t0 * 128 + r * rows
            out[dst0:dst0 + rows] = res[src0:src0 + rows]
    return out



# revision 2
# speedup vs baseline: 1.5842x; 1.5842x over previous
"""Trainium2 Bass kernel for GQA attention (B=2, S=1024, HID=4096, H=32,
HKV=8, HD=128) with NeoX rotary + additive mask, sharded over 8 NeuronCores.

Sharding: 2 data-parallel groups (one per batch sequence) x 4-way tensor
parallel (8 q-heads / 2 kv-heads per core). wq/wk/wv column-sharded.

Output projection: instead of row-sharded wo + ReduceScatter(add) after it
(which puts a multi-10us collective serially at the end of the kernel), each
core AllGathers the group's attention outputs (2.1 MB/rank, bypass) as soon
as each 512-token block finishes attention, then applies a COLUMN shard of wo
against the full gathered activations. The AG for token-block qb1 flies under
qb0's attention compute; the AG for qb0 flies under qb1's wo matmuls; outputs
DMA straight to the external tensor, so no collective is exposed.

Everything on device runs in a transposed layout ([feature, token]) so every
matmul streams with free-dim 512 at full PE rate (bf16 operands everywhere,
fp32 PSUM accumulation).
"""

import math

import ml_dtypes
import numpy as np

B, S, HID, H, HKV, HD = 2, 1024, 4096, 32, 8, 128
NCORES = 8
TPG = 4                      # tensor-parallel group size
NGROUPS = NCORES // TPG      # data-parallel groups (= B)
HL = H // TPG                # q heads per core (8)
KVL = HKV // TPG             # kv heads per core (2)
GQ = H // HKV                # q heads per kv head (4)
SCALE = 1.0 / math.sqrt(HD)
QB = 512                     # q block (free dim of attention matmuls)
NQB = S // QB
NEG_THRESH = -1.0e8          # mask values <= this count as fully masked
OC = HL * HD                 # output columns per core (1024)

_STATE: dict = {}


# ----------------------------------------------------------------------------
# walrus compat: this toolchain supports at most ONE semaphore wait per
# instruction; Tile's scheduler can attach several. Hoist extras onto
# same-engine nops placed immediately before the instruction.
# ----------------------------------------------------------------------------
def _split_multi_waits(nc):
    import concourse.mybir as mybir

    def detached_nop(engine_type):
        bi = nc.engines[engine_type].nop()
        inst = bi.ins
        for fn in nc.m.functions:
            for b in fn.blocks:
                il = b.instructions
                if il and il[-1].name == inst.name:
                    il.pop()
                    return inst
        raise AssertionError("could not detach nop")

    for fn in nc.m.functions:
        for b in fn.blocks:
            il = b.instructions
            out = []
            changed = False
            for inst in il:
                si = inst.sync_info
                waits = list(si.on_wait) if (si is not None and si.on_wait) else []
                if len(waits) > 1:
                    for w in waits[:-1]:
                        nop = detached_nop(inst.engine)
                        nop.sync_info = mybir.SyncInfo(on_wait=[w], on_update=[])
                        out.append(nop)
                    si.on_wait = waits[-1:]
                    changed = True
                out.append(inst)
            if changed:
                b.instructions = out


# ----------------------------------------------------------------------------
# Device program
# ----------------------------------------------------------------------------
def _build_module(mask_desc):
    """mask_desc: per (qb, kb) block descriptor list computed on the host from
    the actual attn_mask:
      ("skip",)                 block fully masked
      ("full", need_mask:bool)  full 512-wide block, optionally + mask data
      ("causal", off:int)       causal window: cols [off,512) active, mask
                                add on the 128-wide diagonal window at `off`
    """
    import concourse.bass as bass
    import concourse.mybir as mybir
    import concourse.tile as tile
    from concourse.masks import make_identity

    dt = mybir.dt
    f32, bf16 = dt.float32, dt.bfloat16
    KT = HID // 128  # 32 contraction tiles

    nc = bass.Bass()

    # --- DRAM parameters (per-core shards, host-prepared) ---
    xt_in = nc.declare_dram_parameter("xt", [S // QB, KT, 128, QB], bf16,
                                      isOutput=False)
    wq_in = nc.declare_dram_parameter("wq", [HL, 128, KT, 128], bf16, isOutput=False)
    wk_in = nc.declare_dram_parameter("wk", [KVL, 128, KT, 128], bf16, isOutput=False)
    wv_in = nc.declare_dram_parameter("wv", [KVL, 128, KT, 128], bf16, isOutput=False)
    # column shard of wo, pre-transposed: [p, kt, col]
    wo_in = nc.declare_dram_parameter("wo", [128, KT, OC], bf16, isOutput=False)
    cos_in = nc.declare_dram_parameter("cos_t", [128, S], f32, isOutput=False)
    sin_in = nc.declare_dram_parameter("sin_t", [128, S], f32, isOutput=False)
    # mask blocks actually referenced by the program, in transposed [kv, q]
    # layout; index map built below.
    mask_tiles = []
    for qb in range(NQB):
        for kb in range(S // 128):
            d = mask_desc[qb][kb]
            if d[0] == "full" and d[1]:
                mask_tiles.append((qb, kb, QB))
    nmask = max(1, len(mask_tiles))
    mw = max([t[2] for t in mask_tiles], default=128)
    mask_in = nc.declare_dram_parameter("maskt", [nmask, 128, mw], f32, isOutput=False)
    tri_in = nc.declare_dram_parameter("tri01", [128, 128], bf16, isOutput=False)
    out_ext = nc.declare_dram_parameter("outp", [S, OC], bf16, isOutput=True)

    from contextlib import ExitStack
    ctx = ExitStack()
    with tile.TileContext(nc) as tc:
        const = ctx.enter_context(tc.tile_pool(name="const", bufs=1))
        persist = ctx.enter_context(tc.tile_pool(name="persist", bufs=1))
        dram = ctx.enter_context(tc.tile_pool(name="dram", bufs=1, space="DRAM"))
        qkvpool = ctx.enter_context(tc.tile_pool(name="qkv", bufs=1))

        # AllGather staging: per token-block, own heads out + gathered in
        ag_in = [dram.tile([HL * 128, QB], bf16, name=f"ag_in{qb}")
                 for qb in range(NQB)]
        ag_out = [dram.tile([KT, 128, QB], bf16, name=f"ag_out{qb}")
                  for qb in range(NQB)]

        ones32 = const.tile([128, 128], f32, tag="ones32")
        nc.gpsimd.memset(ones32[:], 1.0)
        ones_t = const.tile([128, 128], bf16, tag="ones")
        nc.vector.tensor_copy(ones_t[:], ones32[:])
        ident = const.tile([128, 128], f32, tag="ident")
        make_identity(nc, ident[:])

        # activations that live through attention (freed before wo)
        q_rot = [qkvpool.tile([128, S], bf16, tag=f"q{h}", name=f"q_rot{h}")
                 for h in range(HL)]
        k_rot = [qkvpool.tile([128, S], bf16, tag=f"k{j}", name=f"k_rot{j}")
                 for j in range(KVL)]
        v_nat = [qkvpool.tile([128, S // 128, 128], bf16, tag=f"v{j}", name=f"v_nat{j}")
                 for j in range(KVL)]
        attn = persist.tile([128, HL, S], bf16, tag="attn")

        # ------ phase 1 (k/v/q projections) ------
        with tc.tile_pool(name="p1x", bufs=1) as xpool, \
             tc.tile_pool(name="p1w", bufs=3) as wpool, \
             tc.tile_pool(name="p1t", bufs=2) as tpool, \
             tc.tile_pool(name="p1ps", bufs=2, space="PSUM") as pspool:

            # first weight tile DMA'd before x so the first chain's lhsT is
            # never the long pole
            w0_sb = wpool.tile([128, KT, 128], bf16, tag="w")
            nc.sync.dma_start(out=w0_sb[:], in_=wk_in[0])

            # one tile per (tb, kt-chunk) so the first projection chain only
            # waits on the tb=0 quarter-chunks, in DMA issue order.
            xt = [[xpool.tile([128, KT // 4, QB], bf16, tag=f"xt{tb}_{i}",
                              name=f"xt{tb}_{i}") for i in range(4)]
                  for tb in range(NQB)]
            for tb in range(NQB):
                for i in range(4):
                    nc.sync.dma_start(
                        out=xt[tb][i][:],
                        in_=xt_in[tb, i * (KT // 4):(i + 1) * (KT // 4), :, :]
                            .rearrange("k p t -> p k t"),
                    )

            # rope tables + mask/tri constants, needed only ~1 chain in
            cos_t = const.tile([128, S], f32, tag="cos")
            sin_t = const.tile([128, S], f32, tag="sin")
            nc.sync.dma_start(out=cos_t[:], in_=cos_in[:])
            nc.sync.dma_start(out=sin_t[:], in_=sin_in[:])
            mask_sb = const.tile([128, nmask, mw], f32, tag="mask")
            nc.sync.dma_start(out=mask_sb[:],
                              in_=mask_in[:].rearrange("b p c -> p b c"))
            mask_idx = {(qb, kb): i for i, (qb, kb, _) in enumerate(mask_tiles)}
            tri01 = const.tile([128, 128], bf16, tag="tri01")
            nc.sync.dma_start(out=tri01[:], in_=tri_in[:])

            def xt_sl(kt, tb):
                return xt[tb][kt // (KT // 4)][:, kt % (KT // 4), :]

            w_drams = {"k": wk_in, "v": wv_in, "q": wq_in}
            w_tiles = {}

            def chain(kind, ct, tb, pstr=None):
                if tb == 0:
                    if kind == "k" and ct == 0:
                        w_tiles["cur"] = w0_sb
                    else:
                        t = wpool.tile([128, KT, 128], bf16, tag="w",
                                       name=f"w_{kind}{ct}")
                        nc.sync.dma_start(out=t[:], in_=w_drams[kind][ct])
                        w_tiles["cur"] = t
                w_sb = w_tiles["cur"]
                ps = pspool.tile([128, QB], f32, tag="ps_qkv")
                for kt in range(KT):
                    nc.tensor.matmul(
                        ps[:],
                        w_sb[:, kt, :],
                        xt_sl(kt, tb),
                        start=(kt == 0),
                        stop=(kt == KT - 1),
                    )
                tsl = slice(tb * QB, (tb + 1) * QB)
                if kind in ("q", "k"):
                    dest = q_rot[ct] if kind == "q" else k_rot[ct]
                    swap = tpool.tile([128, QB], f32, tag="swap")
                    nc.scalar.activation(
                        swap[0:64, :], ps[64:128, :],
                        mybir.ActivationFunctionType.Copy, scale=-1.0)
                    nc.scalar.activation(
                        swap[64:128, :], ps[0:64, :],
                        mybir.ActivationFunctionType.Copy)
                    t2 = tpool.tile([128, QB], f32, tag="t2")
                    nc.vector.tensor_tensor(
                        t2[:], ps[:], cos_t[:, tsl], mybir.AluOpType.mult)
                    t3 = tpool.tile([128, QB], f32, tag="t3")
                    nc.vector.tensor_tensor(
                        t3[:], swap[:], sin_t[:, tsl], mybir.AluOpType.mult)
                    nc.vector.tensor_tensor(
                        dest[:, tsl], t2[:], t3[:], mybir.AluOpType.add)
                else:  # v: transpose to natural [t, d] layout
                    vt = tpool.tile([128, QB], f32, tag="vt")
                    nc.scalar.activation(
                        vt[:], ps[:], mybir.ActivationFunctionType.Copy)
                    for j in range(QB // 128):
                        ps_t = pstr.tile([128, 128], f32, tag="ps_tr")
                        nc.tensor.transpose(
                            ps_t[:], vt[:, j * 128:(j + 1) * 128], ident[:])
                        nc.vector.tensor_copy(
                            v_nat[ct][:, tb * (QB // 128) + j, :], ps_t[:])

            # k/v chains first (tb-interleaved so x-feed stalls stay short)
            with tc.tile_pool(name="p1pst", bufs=2, space="PSUM") as pstr:
                for kind, n in (("k", KVL), ("v", KVL)):
                    for ct in range(n):
                        for tb in range(NQB):
                            chain(kind, ct, tb, pstr)

            # q chains (plain)
            for ct in range(HL):
                for tb in range(NQB):
                    chain("q", ct, tb)

        # ------- attention machinery -------
        ppool = ctx.enter_context(tc.tile_pool(name="p2p", bufs=3))
        rpool = ctx.enter_context(tc.tile_pool(name="p2r", bufs=2))

        def head_blocks(qb):
            blocks = []
            for kb in range(S // 128):
                d = mask_desc[qb][kb]
                if d[0] == "skip":
                    continue
                if d[0] == "causal":
                    blocks.append((kb, d[1], ("diag", d[1])))
                else:
                    blocks.append((kb, 0, ("full",) if d[1] else None))
            return blocks

        def make_attention(qb, scpool, pvpool, dnpool):
            blocks = head_blocks(qb)
            nblk = len(blocks)
            state = {}  # h -> (ps_pv, ps_dn, p_ts)

            def issue_score(h, bi):
                kvh = h // GQ
                kb, off, mk = blocks[bi]
                qsl = slice(qb * QB + off, (qb + 1) * QB)
                ps_sc = scpool.tile([128, QB], f32, tag="ps_sc")
                nc.tensor.matmul(
                    ps_sc[:, off:QB],
                    k_rot[kvh][:, kb * 128:(kb + 1) * 128],
                    q_rot[h][:, qsl],
                    start=True, stop=True,
                )
                if mk is not None and mk[0] != "diag":
                    # rare general path: additive mask on PSUM via DVE
                    mi = mask_idx[(qb, kb)]
                    nc.vector.tensor_tensor(
                        ps_sc[:, 0:QB], ps_sc[:, 0:QB],
                        mask_sb[:, mi, 0:QB], mybir.AluOpType.add)
                p_t = ppool.tile([128, QB], bf16, tag="p")
                nc.scalar.activation(
                    p_t[:, off:QB], ps_sc[:, off:QB],
                    mybir.ActivationFunctionType.Exp)
                if mk is not None and mk[0] == "diag":
                    # causal diagonal: zero the kv>q triangle of exp(s) in
                    # SBUF on the vector engine (Pool is reserved for the
                    # collective queue so its waits never gate compute)
                    nc.vector.tensor_tensor(
                        p_t[:, off:off + 128], p_t[:, off:off + 128],
                        tri01[:], mybir.AluOpType.mult)
                if h not in state:
                    ps_pv = pvpool.tile([128, QB], f32, tag="ps_pv",
                                        name=f"ps_pv{qb}_{h}")
                    ps_dn = dnpool.tile([128, QB], f32, tag="ps_dn",
                                        name=f"ps_dn{qb}_{h}")
                    state[h] = (ps_pv, ps_dn, {})
                state[h][2][bi] = p_t

            def issue_pv_dn(h, bi):
                kvh = h // GQ
                kb, off, mk = blocks[bi]
                ps_pv, ps_dn, p_ts = state[h]
                p_t = p_ts.pop(bi)
                nc.tensor.matmul(
                    ps_pv[:, off:QB],
                    v_nat[kvh][:, kb, :],
                    p_t[:, off:QB],
                    start=(bi == 0), stop=(bi == nblk - 1),
                )
                nc.tensor.matmul(
                    ps_dn[:, off:QB],
                    ones_t[:],
                    p_t[:, off:QB],
                    start=(bi == 0), stop=(bi == nblk - 1),
                )
                if bi == nblk - 1:
                    recip = rpool.tile([128, QB], f32, tag="recip")
                    nc.vector.reciprocal(recip[:], ps_dn[:])
                    nc.vector.tensor_tensor(
                        attn[:, h, qb * QB:(qb + 1) * QB], ps_pv[:],
                        recip[:], mybir.AluOpType.mult)

            return nblk, issue_score, issue_pv_dn

        def attention_phase(qb, scp, pvp, dnp):
            nblk, score, pvdn = make_attention(qb, scp, pvp, dnp)
            events = [(h, bi) for h in range(HL) for bi in range(nblk)]
            LOOKAHEAD = 1
            n = len(events)
            for j in range(min(LOOKAHEAD, n)):
                score(*events[j])
            for j in range(LOOKAHEAD, n):
                score(*events[j])
                pvdn(*events[j - LOOKAHEAD])
            for j in range(max(0, n - LOOKAHEAD), n):
                pvdn(*events[j])

        # wo shard prefetched under qb1's attention
        wopool = ctx.enter_context(tc.tile_pool(name="p23w", bufs=1))
        opool = ctx.enter_context(tc.tile_pool(name="p3o", bufs=4))
        agpool = ctx.enter_context(tc.tile_pool(name="p3ag", bufs=2))
        wo_sb = wopool.tile([128, KT, OC], bf16, tag="wo")
        nc.sync.dma_start(out=wo_sb[:], in_=wo_in[:])

        attg = {}

        def emit_ag(qb):
            # own heads -> DRAM (SP queue: waits only on attention output)
            for h in range(HL):
                nc.sync.dma_start(
                    out=ag_in[qb][h * 128:(h + 1) * 128, :],
                    in_=attn[:, h, qb * QB:(qb + 1) * QB])
            # gather across the TP group (collective queue; overlaps compute)
            nc.gpsimd.collective_compute(
                "AllGather", mybir.AluOpType.bypass,
                replica_groups=[list(range(g * TPG, (g + 1) * TPG))
                                for g in range(NGROUPS)],
                ins=[ag_in[qb][:].opt()],
                outs=[ag_out[qb][:].opt()],
            )
            # gathered activations -> SBUF on the SWDGE/Pool queue so the
            # wait on the collective never blocks compute-critical DMAs
            attg[qb] = agpool.tile([128, KT, QB], bf16, tag="attg",
                                   name=f"attg{qb}")
            nc.gpsimd.dma_start(
                out=attg[qb][:],
                in_=ag_out[qb][:].rearrange("k p t -> p k t"))

        def wo_phase(qb, psp):
            a = attg[qb]
            for tt4 in range(4):
                row0 = qb * QB + tt4 * 128
                for hb in range(OC // QB):
                    ps_o = psp.tile([128, QB], f32, tag="ps_o")
                    for kt in range(KT):
                        nc.tensor.matmul(
                            ps_o[:],
                            a[:, kt, tt4 * 128:(tt4 + 1) * 128],
                            wo_sb[:, kt, hb * QB:(hb + 1) * QB],
                            start=(kt == 0), stop=(kt == KT - 1),
                        )
                    o_sb = opool.tile([128, QB], bf16, tag="o")
                    if (hb + tt4) % 2 == 0:
                        nc.vector.tensor_copy(o_sb[:], ps_o[:])
                    else:
                        nc.scalar.activation(
                            o_sb[:], ps_o[:], mybir.ActivationFunctionType.Copy)
                    nc.sync.dma_start(
                        out=out_ext[row0:row0 + 128, hb * QB:(hb + 1) * QB],
                        in_=o_sb[:])

        # qb1 first: its AG flies under qb0's attention; qb0's AG flies
        # under qb1's wo matmuls.
        with tc.tile_pool(name="p2sc1", bufs=3, space="PSUM") as scp, \
             tc.tile_pool(name="p2pv1", bufs=2, space="PSUM") as pvp, \
             tc.tile_pool(name="p2dn1", bufs=2, space="PSUM") as dnp:
            attention_phase(1, scp, pvp, dnp)
        emit_ag(1)
        with tc.tile_pool(name="p2sc0", bufs=3, space="PSUM") as scp, \
             tc.tile_pool(name="p2pv0", bufs=2, space="PSUM") as pvp, \
             tc.tile_pool(name="p2dn0", bufs=2, space="PSUM") as dnp:
            attention_phase(0, scp, pvp, dnp)
        emit_ag(0)
        with tc.tile_pool(name="p3ps1", bufs=3, space="PSUM") as psp:
            wo_phase(1, psp)
        with tc.tile_pool(name="p3ps0", bufs=3, space="PSUM") as psp:
            wo_phase(0, psp)

        ctx.close()

    _split_multi_waits(nc)
    return nc, [t[:2] for t in mask_tiles], mw


# ----------------------------------------------------------------------------
# Host-side input prep
# ----------------------------------------------------------------------------
def _classify_mask(attn_mask):
    """Per (qb, kb) descriptor from the actual mask contents (transposed
    [kv, q] view). Causal masks produce the efficient windowed structure."""
    mt = attn_mask.T  # [kv, q]
    desc = []
    for qb in range(S // QB):
        row = []
        q0 = qb * QB
        for kb in range(S // 128):
            blk = mt[kb * 128:(kb + 1) * 128, q0:q0 + QB]
            if np.all(blk <= NEG_THRESH):
                row.append(("skip",))
                continue
            if np.all(np.abs(blk) < 1e-6):
                row.append(("full", False))
                continue
            # causal window? cols [0, off) fully masked, diag at [off, off+128),
            # cols beyond fully visible
            off = kb * 128 - q0
            causal = False
            if 0 <= off <= QB - 128:
                left_ok = np.all(blk[:, :off] <= NEG_THRESH) if off else True
                right_ok = (np.all(np.abs(blk[:, off + 128:]) < 1e-6)
                            if off + 128 < QB else True)
                causal = bool(left_ok and right_ok)
            if causal:
                row.append(("causal", off))
            else:
                row.append(("full", True))
        desc.append(row)
    # every q column must keep at least one contributing block
    for qb in range(S // QB):
        assert any(d[0] != "skip" for d in desc[qb]), "fully-masked q rows unsupported"
    return desc


def _prep_core_inputs(inputs, mask_desc, mask_list, mw):
    x = np.asarray(inputs["x"], np.float32)
    wq = np.asarray(inputs["wq"], np.float32)
    wk = np.asarray(inputs["wk"], np.float32)
    wv = np.asarray(inputs["wv"], np.float32)
    wo = np.asarray(inputs["wo"], np.float32)
    attn_mask = np.asarray(inputs["attn_mask"], np.float32)
    start_pos = np.asarray(inputs["start_pos"], np.int32)

    bf = ml_dtypes.bfloat16
    KT = HID // 128

    inv_freq = 1.0 / (10000.0 ** (np.arange(0, HD, 2, dtype=np.float32) / HD))
    mt = attn_mask.T
    if mask_list:
        mask_arr = np.zeros((len(mask_list), 128, mw), np.float32)
        for i, (qb, kb) in enumerate(mask_list):
            mask_arr[i, :, 0:QB] = mt[kb * 128:(kb + 1) * 128,
                                      qb * QB:(qb + 1) * QB]
    else:
        mask_arr = np.zeros((1, 128, mw), np.float32)
    # 0/1 lower-triangle (kv <= q) pattern shared by every causal diag block
    tri01 = (np.arange(128)[:, None] <= np.arange(128)[None, :]).astype(bf)

    # lhsT tile layout: [ct, p=hid_within_kt, kt, col_within_ct]
    def wtile2(w):
        c = w.shape[1]
        return np.ascontiguousarray(
            w.reshape(KT, 128, c // 128, 128).transpose(2, 1, 0, 3))

    in_maps = []
    for core in range(NCORES):
        g, r = divmod(core, TPG)
        xb = x[g * S:(g + 1) * S]                       # [S, HID]
        xt = np.ascontiguousarray(
            xb.T.reshape(KT, 128, S // QB, QB).transpose(2, 0, 1, 3)).astype(bf)
        wq_c = (wq[:, r * HL * HD:(r + 1) * HL * HD] * SCALE)
        wk_c = wk[:, r * KVL * HD:(r + 1) * KVL * HD]
        wv_c = wv[:, r * KVL * HD:(r + 1) * KVL * HD]
        # COLUMN shard of wo, laid out [p, kt, col] for a single linear DMA
        wo_c = wo[:, r * OC:(r + 1) * OC]               # [HID, OC]
        wo_t = np.ascontiguousarray(
            wo_c.reshape(KT, 128, OC).transpose(1, 0, 2))

        pos = start_pos[g] + np.arange(S, dtype=np.float32)
        ang = pos[:, None] * inv_freq[None, :]          # [S, HD/2]
        cos = np.concatenate([np.cos(ang), np.cos(ang)], -1).T  # [HD, S]
        sin = np.concatenate([np.sin(ang), np.sin(ang)], -1).T

        in_maps.append({
            "xt": xt,
            "wq": wtile2(wq_c).astype(bf),
            "wk": wtile2(wk_c).astype(bf),
            "wv": wtile2(wv_c).astype(bf),
            "wo": wo_t.astype(bf),
            "cos_t": np.ascontiguousarray(cos.astype(np.float32)),
            "sin_t": np.ascontiguousarray(sin.astype(np.float32)),
            "maskt": mask_arr,
            "tri01": tri01,
        })
    return in_maps


def _make_runner(nc):
    """Cached jit over the bass module (adapted from
    concourse.bass2jax.run_bass_via_pjrt so repeat calls reuse one NEFF)."""
    import jax
    import jax.numpy as jnp
    from jax.sharding import Mesh, NamedSharding, PartitionSpec
    from jax.experimental.shard_map import shard_map

    import concourse.mybir as mybir
    from concourse import bass2jax

    bass2jax.install_neuronx_cc_hook()
    assert nc.dbg_addr is None
    partition_name = (nc.partition_id_tensor.name
                      if nc.partition_id_tensor else None)

    in_names, out_names, out_avals, out_shapes = [], [], [], []
    for alloc in nc.m.functions[0].allocations:
        if not isinstance(alloc, mybir.MemoryLocationSet):
            continue
        name = alloc.memorylocations[0].name
        if alloc.kind == "ExternalInput":
            if name != partition_name:
                in_names.append(name)
        elif alloc.kind == "ExternalOutput":
            assert alloc.tensor_shape is not None and alloc.dtype is not None
            shape = tuple(alloc.tensor_shape)
            npdt = mybir.dt.np(alloc.dtype)
            out_names.append(name)
            out_shapes.append((shape, npdt))
            out_avals.append(jax.core.ShapedArray(shape, npdt))

    n_params = len(in_names)
    n_outs = len(out_names)
    all_in_names = in_names + out_names
    if partition_name is not None:
        all_in_names = all_in_names + [partition_name]
    donate = tuple(range(n_params, n_params + n_outs))

    def _body(*args):
        operands = list(args)
        if partition_name is not None:
            operands.append(bass2jax.partition_id_tensor())
        outs = bass2jax._bass_exec_p.bind(
            *operands,
            out_avals=tuple(out_avals),
            in_names=tuple(all_in_names),
            out_names=tuple(out_names),
            lowering_input_output_aliases=(),
            sim_require_finite=True,
            sim_require_nnan=True,
            nc=nc,
        )
        return tuple(outs)

    devices = jax.devices()[:NCORES]
    mesh = Mesh(np.asarray(devices), ("core",))
    pc = PartitionSpec("core")
    sharded = jax.jit(
        shard_map(_body, mesh=mesh, in_specs=(pc,) * (n_params + n_outs),
                  out_specs=(pc,) * n_outs, check_rep=False),
        donate_argnums=donate, keep_unused=True)

    shard_dev = NamedSharding(mesh, pc)

    def make_zeros():
        return tuple(
            jax.device_put(np.zeros((NCORES * s[0], *s[1:]), d), shard_dev)
            for s, d in out_shapes)

    def put_inputs(in_maps):
        return [
            jax.device_put(
                np.concatenate([np.asarray(m[nm]) for m in in_maps], axis=0),
                shard_dev)
            for nm in in_names]

    def run_from_dev(in_dev, zeros):
        out_arrs = sharded(*in_dev, *zeros)
        jax.block_until_ready(out_arrs)
        return out_arrs

    def run(in_maps):
        out_arrs = run_from_dev(put_inputs(in_maps), make_zeros())
        return [
            {nm: np.asarray(out_arrs[i]).reshape(NCORES, *out_shapes[i][0])[c]
             for i, nm in enumerate(out_names)}
            for c in range(NCORES)]

    return {"run": run, "put_inputs": put_inputs, "make_zeros": make_zeros,
            "run_from_dev": run_from_dev, "sharded": sharded}


def _get_runner(mask_desc):
    key = repr(mask_desc)
    if _STATE.get("key") == key:
        return _STATE["run"], _STATE["mask_list"], _STATE["mw"]

    nc, mask_list, mw = _build_module(mask_desc)
    runner = _make_runner(nc)

    _STATE.update({"key": key, "run": runner["run"], "mask_list": mask_list,
                   "mw": mw, "nc": nc, "runner": runner})
    return runner["run"], mask_list, mw


def kernel(**inputs) -> np.ndarray:
    attn_mask = np.asarray(inputs["attn_mask"], np.float32)
    mask_desc = _classify_mask(attn_mask)
    run, mask_list, mw = _get_runner(mask_desc)
    in_maps = _prep_core_inputs(inputs, mask_desc, mask_list, mw)
    results = run(in_maps)
    out = np.empty((B * S, HID), np.float32)
    for core in range(NCORES):
        g, r = divmod(core, TPG)
        res = np.asarray(results[core]["outp"], np.float32)  # [S, OC]
        out[g * S:(g + 1) * S, r * OC:(r + 1) * OC] = res
    return out
